# revision 23
# baseline (speedup 1.0000x reference)
"""Trainium2 Bass kernel for nn_DSTGCM (dynamic spatio-temporal graph conv).

Per-core SPMD: node-shard NS=256 of N=2048, all BT=16 (b,t) pairs.
v2: host-side repack for big DMA descriptors (x as fp16 [bt,p,mc,c] tiles,
pre-transposed ne/ne_sh/x_sh); LN stats via ones-vector matmuls on PE;
QK with shared stationary at N=512 (2 bt per matmul, fp32r full rate);
PV and apply in fp16; Pool engine offload for elementwise; x fully
prefetched to SBUF so the main loop issues no DMAs.
"""

import sys
import numpy as np
from contextlib import ExitStack

for _p in ("/opt/trn_rl_repo",):
    if _p not in sys.path:
        sys.path.insert(0, _p)

import concourse.bass as bass
import concourse.bacc as bacc
import concourse.mybir as mybir
import concourse.tile as tile
from concourse import masks
from concourse.bass_utils import run_bass_kernel_spmd

F32 = mybir.dt.float32
F32R = mybir.dt.float32r
F16 = mybir.dt.float16
AF = mybir.ActivationFunctionType
ALU = mybir.AluOpType

N_CORES = 8
B, T, NFULL, DI, DO, D = 8, 2, 2048, 64, 64, 64
BT = B * T
LN_EPS = 1e-12


def r(ap):
    return ap.bitcast(mybir.dt.float32r)


def build_program(N=NFULL, NS=NFULL // N_CORES, bt=BT, num_devices=N_CORES):
    KIO = 2 * DI * DO  # 8192
    NT = N // 128      # 16 m-chunks
    XW = NT * (DI + 1)  # 1040 free per x tile

    nc = bacc.Bacc("TRN2", target_bir_lowering=False, debug=False,
                   num_devices=num_devices)

    # DRAM inputs (host-repacked)
    x_r = nc.dram_tensor("x_r", [bt * 128, XW], F16, kind="ExternalInput").ap()
    xshT = nc.dram_tensor("xshT", [DI, bt * NS], F16, kind="ExternalInput").ap()
    neT = nc.dram_tensor("neT", [D, N], F32R, kind="ExternalInput").ap()
    neTs = nc.dram_tensor("neTs", [D, NS], F32R, kind="ExternalInput").ap()
    te = nc.dram_tensor("te", [bt, D], F32, kind="ExternalInput").ap()
    wp = nc.dram_tensor("wp", [D, KIO], F32R, kind="ExternalInput").ap()
    bp = nc.dram_tensor("bp", [D, DO], F32R, kind="ExternalInput").ap()
    out_d = nc.dram_tensor("out_sh", [bt * NS, DO], F32, kind="ExternalOutput").ap()

    with tile.TileContext(nc) as tc, ExitStack() as ctx:
        cst = ctx.enter_context(tc.tile_pool(name="cst", bufs=1))
        big = ctx.enter_context(tc.tile_pool(name="big", bufs=1))

        ident = cst.tile([128, 128], F32)
        masks.make_identity(nc, ident[:])
        neg64 = cst.tile([128, 1], F32)
        nc.vector.memset(neg64[:], -64.0)
        ones64 = cst.tile([64, 1], F32)
        nc.vector.memset(ones64[:], 1.0)

        # persistent SBUF tensors
        xs = big.tile([128, bt * XW], F16)          # all x, [p, (b, mc, c)]
        xs_v = xs[:].rearrange("p (b w) -> p b w", b=bt)
        neTt = big.tile([64, N], F32R)               # raw ne^T
        neTst = cst.tile([64, NS], F32R)             # raw ne_sh^T
        necT = cst.tile([66, N], F32R)               # [0:64] centered, 64 ones, 65 var
        necTs = cst.tile([66, NS], F32R)
        tecT = cst.tile([64, bt], F32)
        teT = cst.tile([64, bt], F32R)
        preVarT = cst.tile([66, bt], F32R)
        ckT = cst.tile([65, bt], F32R)
        rstdF = cst.tile([16, N], F32)
        rstdS = cst.tile([16, NS], F32)
        rstdT = cst.tile([128, NT * 16], F32)
        crowk = cst.tile([16, NS], F32)
        wpS = big.tile([64, KIO], F32R)
        bpS = cst.tile([64, DO], F32R)
        teS = cst.tile([16, D], F32)
        Wsb = big.tile([128, DO * NS], F16)         # free (o, n)
        W_v = Wsb[:].rearrange("ki (o n) -> ki o n", o=DO)
        xgT = big.tile([128, bt * NS], F16)         # free (b, n)
        xgT_v = xgT[:].rearrange("ki (b n) -> ki b n", b=bt)
        zrow = big.tile([1, bt * NS], F16)          # softmax denominators
        ztt = cst.tile([16, NS], F16)               # Z gathered to partitions
        rzt = cst.tile([16, NS], F32)               # 1/Z
        biasR = cst.tile([16, 8 * DO], F32)

        # ---- prefetch all DMAs (prep tensors first) ----
        nc.sync.dma_start(neTt[:], neT)
        nc.sync.dma_start(neTst[:], neTs)
        nc.sync.dma_start(teS[:bt, :], te)
        nc.sync.dma_start(bpS[:], bp)
        for q in range(4):
            nc.scalar.dma_start(wpS[:, q * 2048:(q + 1) * 2048],
                                wp[:, q * 2048:(q + 1) * 2048])
        nc.sync.dma_start(xgT[0:64, :], xshT)
        for b_ in range(bt):
            nc.gpsimd.dma_start(xs_v[:, b_, :], x_r[b_ * 128:(b_ + 1) * 128, :])

        with tc.tile_pool(name="pp", bufs=4, space="PSUM") as pp, \
             tc.tile_pool(name="wk", bufs=1) as wk:
            # ---- te stats (tiny) ----
            mus = cst.tile([16, 1], F32)
            nc.vector.reduce_sum(mus[:bt], teS[:bt, :], axis=mybir.AxisListType.X)
            mu = cst.tile([16, 1], F32)
            nc.vector.tensor_scalar_mul(mu[:bt], mus[:bt], 1.0 / D)
            tec = cst.tile([16, D], F32)
            nc.vector.tensor_scalar_sub(tec[:bt], teS[:bt, :], mu[:bt])
            sq = cst.tile([16, D], F32)
            kap = cst.tile([16, 1], F32)
            nc.vector.tensor_tensor(out=sq[:bt], in0=tec[:bt], in1=tec[:bt],
                                    op=ALU.mult)
            nc.vector.reduce_sum(kap[:bt], sq[:bt], axis=mybir.AxisListType.X)

            pv = cst.tile([16, 66], F32)
            nc.vector.tensor_scalar_mul(pv[:bt, 0:64], tec[:bt], 2.0 / D)
            nc.vector.tensor_scalar_mul(pv[:bt, 64:65], kap[:bt], 1.0 / D)
            nc.vector.memset(pv[:bt, 65:66], 1.0)
            pps = pp.tile([66, 16], F32, tag="pp")
            nc.tensor.transpose(pps[:, :bt], pv[:bt, :], ident[:bt, :bt])
            nc.scalar.copy(preVarT[:, :bt], pps[:, :bt])

            pc = cst.tile([16, 65], F32)
            nc.vector.tensor_copy(pc[:bt, 0:64], tec[:bt])
            nc.vector.tensor_copy(pc[:bt, 64:65], kap[:bt])
            pps2 = pp.tile([65, 16], F32, tag="pp")
            nc.tensor.transpose(pps2[:, :bt], pc[:bt, :], ident[:bt, :bt])
            nc.scalar.copy(ckT[:, :bt], pps2[:, :bt])

            pps3 = pp.tile([64, 16], F32, tag="pp")
            nc.tensor.transpose(pps3[:, :bt], tec[:bt, :], ident[:bt, :bt])
            nc.scalar.copy(tecT[:, :bt], pps3[:, :bt])
            pps4 = pp.tile([64, 16], F32, tag="pp")
            nc.tensor.transpose(pps4[:, :bt], teS[:bt, :], ident[:bt, :bt])
            nc.scalar.copy(teT[:, :bt], pps4[:, :bt])

            # ---- ne stats via ones-matmuls (full + shard) ----
            def ne_stats(srct, ncols, dst):
                src = srct[:]
                # mean
                mrow = wk.tile([1, ncols], F32, tag="sC" + str(ncols))
                for jj in range(0, ncols, 512):
                    w = min(512, ncols - jj)
                    pm = pp.tile([1, 512], F32, tag="pp")
                    nc.tensor.matmul(pm[:, :w], r(ones64[:]),
                                     r(src[:, jj:jj + w]), start=True, stop=True)
                    nc.vector.tensor_scalar_mul(mrow[:, jj:jj + w], pm[:, :w],
                                                1.0 / D)
                # center: dst[0:64] = src - mean (bcast over partitions)
                mb = wk.tile([64, ncols], F32, tag="sA" + str(ncols))
                nc.sync.dma_start(
                    mb[:], mrow[:].unsqueeze(1).broadcast_to([1, 64, ncols]))
                nc.vector.tensor_tensor(out=dst[0:64, :],
                                        in0=src[:].bitcast(F32),
                                        in1=mb[:], op=ALU.subtract)
                nc.gpsimd.memset(dst[64:65, :].bitcast(F32), 1.0)
                # var = sum(centered^2)/D
                sqc = wk.tile([64, ncols], F32R, tag="sB" + str(ncols))
                nc.scalar.activation(sqc[:], dst[0:64, :].bitcast(F32),
                                     AF.Square)
                vrow = wk.tile([1, ncols], F32R, tag="sC" + str(ncols))
                for jj in range(0, ncols, 512):
                    w = min(512, ncols - jj)
                    pv_ = pp.tile([1, 512], F32, tag="pp")
                    nc.tensor.matmul(pv_[:, :w], r(ones64[:]),
                                     r(sqc[:, jj:jj + w]), start=True, stop=True)
                    nc.vector.tensor_scalar_mul(vrow[:, jj:jj + w],
                                                pv_[:, :w], 1.0 / D)
                nc.sync.dma_start(dst[65:66, :], vrow[:])

            ne_stats(neTt, N, necT)
            ne_stats(neTst, NS, necTs)

            # ---- rstd ----
            def rstd_from(necT_src, ncols, dst):
                t1 = wk.tile([16, ncols], F32, tag="sA" + str(ncols))
                for jj in range(0, ncols, 512):
                    w = min(512, ncols - jj)
                    pvv = pp.tile([16, 512], F32, tag="pp")
                    nc.tensor.matmul(pvv[:bt, :w], r(preVarT[:, :bt]),
                                     r(necT_src[:, jj:jj + w]), start=True,
                                     stop=True)
                    nc.vector.tensor_scalar_add(t1[:bt, jj:jj + w], pvv[:bt, :w],
                                                LN_EPS)
                t2 = wk.tile([16, ncols], F32, tag="sB" + str(ncols))
                nc.vector.reciprocal(t2[:bt], t1[:bt])
                nc.scalar.activation(dst[:bt, :], t2[:bt], AF.Sqrt)

            rstd_from(necT, N, rstdF)
            rstd_from(necTs, NS, rstdS)

            pck = pp.tile([16, NS], F32, tag="pp")
            nc.tensor.matmul(pck[:bt, :], r(ckT[:, :bt]), r(necTs[0:65, :]),
                             start=True, stop=True)
            nc.scalar.copy(crowk[:bt, :], pck[:bt, :])

            for i in range(NT):
                pt = pp.tile([128, 16], F32, tag="pp")
                nc.tensor.transpose(pt[:, :bt], rstdF[:bt, i * 128:(i + 1) * 128],
                                    ident[:bt, :bt])
                nc.scalar.copy(rstdT[:, i * 16:i * 16 + bt], pt[:, :bt])

            # bias
            pb = pp.tile([16, DO], F32, tag="pp")
            nc.tensor.matmul(pb[:bt, :], r(teT[:, :bt]), r(bpS[:, :]),
                             start=True, stop=True)
            bias = cst.tile([16, DO], F32)
            nc.scalar.copy(bias[:bt, :], pb[:bt, :])
            for jj in range(8):
                nc.gpsimd.tensor_copy(biasR[:bt, jj * DO:(jj + 1) * DO],
                                      bias[:bt, :])

        # ---- per-node weights: W[ki, o, n] fp16 ----
        with tc.tile_pool(name="pwp", bufs=4, space="PSUM") as pwp:
            wp_v = wpS[:].rearrange("d (ki o) -> d o ki", o=DO)
            for o in range(DO):
                pw = pwp.tile([128, NS], F32, tag="pw")
                nc.tensor.matmul(pw[:], r(wp_v[:, o, :]), r(neTst[:, :]),
                                 start=True, stop=True)
                if o % 2 == 0:
                    nc.vector.tensor_copy(W_v[:, o, :], pw[:])
                else:
                    nc.scalar.activation(W_v[:, o, :], pw[:], AF.Copy)

        # ---- main loop: 8 bt-pairs x 16 m-chunks ----
        with tc.tile_pool(name="qkp", bufs=4, space="PSUM") as qkp, \
             tc.tile_pool(name="pvp", bufs=2, space="PSUM") as pvp, \
             tc.tile_pool(name="esp", bufs=6) as esp, \
             tc.tile_pool(name="rhp", bufs=3) as rhp:
            for pr in range(bt // 2):
                b0, b1 = 2 * pr, 2 * pr + 1
                prh = rhp.tile([65, 2 * NS], F32, tag="prh")
                pbc = rhp.tile([65, 2 * NS], F32, tag="pbc")
                for h, b_ in enumerate((b0, b1)):
                    sl = slice(h * NS, (h + 1) * NS)
                    nc.vector.tensor_scalar_add(prh[0:64, sl],
                                                necTs[0:64, :].bitcast(F32),
                                                tecT[:, b_:b_ + 1])
                    nc.scalar.dma_start(prh[64:65, sl], crowk[b_:b_ + 1, :])
                    nc.scalar.dma_start(pbc[:, sl],
                                        rstdS[b_:b_ + 1, :].unsqueeze(1)
                                        .broadcast_to([1, 65, NS]))
                rhs = rhp.tile([65, 2 * NS], F32R, tag="rhs")
                nc.vector.tensor_tensor(out=rhs[:], in0=pbc[:], in1=prh[:],
                                        op=ALU.mult)

                ppv0 = pvp.tile([65, NS], F32, tag="ppv0")
                ppv1 = pvp.tile([65, NS], F32, tag="ppv1")
                ppvs = [ppv0, ppv1]
                for mc in range(NT):
                    pS = qkp.tile([128, 2 * NS], F32, tag="pS")
                    nc.tensor.matmul(pS[:],
                                     r(necT[0:65, mc * 128:(mc + 1) * 128]),
                                     r(rhs[:]), start=True, stop=True)
                    es = esp.tile([128, 2 * NS], F16, tag="es")
                    for h, b_ in enumerate((b0, b1)):
                        sl = slice(h * NS, (h + 1) * NS)
                        nc.scalar.activation(
                            es[:, sl], pS[:, sl], AF.Exp, bias=neg64[:],
                            scale=rstdT[:, mc * 16 + b_:mc * 16 + b_ + 1])
                    for h, b_ in enumerate((b0, b1)):
                        sl = slice(h * NS, (h + 1) * NS)
                        xo = xs_v[:, b_, mc * 65:(mc + 1) * 65]
                        nc.tensor.matmul(ppvs[h][:], xo, es[:, sl],
                                         start=(mc == 0), stop=(mc == NT - 1))

                for h, b_ in enumerate((b0, b1)):
                    nc.vector.tensor_copy(zrow[0:1, b_ * NS:(b_ + 1) * NS],
                                          ppvs[h][64:65, :])
                nc.sync.dma_start(
                    ztt[2 * pr:2 * pr + 2, :],
                    zrow[0:1, 2 * pr * NS:(2 * pr + 2) * NS]
                    .rearrange("one (b n) -> one b n", b=2))
                nc.vector.reciprocal(rzt[:bt, :], ztt[:bt, :])
                for h, b_ in enumerate((b0, b1)):
                    rb = rhp.tile([64, NS], F32, tag="rb")
                    nc.sync.dma_start(rb[:], rzt[b_:b_ + 1, :].unsqueeze(1)
                                      .broadcast_to([1, 64, NS]))
                    nc.vector.tensor_tensor(out=xgT_v[64:128, b_, :],
                                            in0=ppvs[h][0:64, :], in1=rb[:],
                                            op=ALU.mult)

        # ---- apply: per-node matmuls, fp16, batched 8 nodes per PSUM tile ----
        out_v = out_d.rearrange("(b n) o -> b (n o)", b=bt)
        with tc.tile_pool(name="pap", bufs=4, space="PSUM") as pap, \
             tc.tile_pool(name="oap", bufs=3) as oap:
            for nb in range(NS // 8):
                pA = pap.tile([16, 8 * DO], F32, tag="pA")
                for jj in range(8):
                    n_ = nb * 8 + jj
                    nc.tensor.matmul(pA[:bt, jj * DO:(jj + 1) * DO],
                                     xgT_v[:, :, n_], W_v[:, :, n_],
                                     start=True, stop=True)
                ob = oap.tile([16, 8 * DO], F32, tag="ob")
                nc.vector.tensor_tensor(out=ob[:bt, :], in0=pA[:bt, :],
                                        in1=biasR[:bt, :], op=ALU.add)
                nc.sync.dma_start(
                    out_v[:, nb * 8 * DO:(nb + 1) * 8 * DO], ob[:bt, :])

    nc.compile()
    return nc


_NC_CACHE = {}


def _get_nc():
    if "nc" not in _NC_CACHE:
        _NC_CACHE["nc"] = build_program()
    return _NC_CACHE["nc"]


def make_in_maps(x, node_embeddings, time_embeddings, weights_pool, bias_pool,
                 N=NFULL, bt=BT, n_cores=N_CORES):
    NS = N // n_cores
    NT = N // 128
    # x_r: [bt, mc, p, c] -> [p, bt, mc, c] -> rows (b*128+p), 1040 cols
    x4 = np.concatenate(
        [x.reshape(bt, N, DI).astype(np.float16),
         np.ones((bt, N, 1), np.float16)], axis=2)      # (bt, N, 65)
    x_r = np.ascontiguousarray(
        x4.reshape(bt, NT, 128, DI + 1).transpose(0, 2, 1, 3)
        .reshape(bt * 128, NT * (DI + 1)))
    ne = node_embeddings.astype(np.float32)
    neT = np.ascontiguousarray(ne.T)                    # (64, N)
    te = np.ascontiguousarray(time_embeddings.reshape(bt, D).astype(np.float32))
    wpf = np.ascontiguousarray(
        weights_pool.reshape(D, 2 * DI * DO).astype(np.float32))
    bpf = np.ascontiguousarray(bias_pool.astype(np.float32))
    x3 = x.reshape(bt, N, DI)
    maps = []
    for c in range(n_cores):
        sl = slice(c * NS, (c + 1) * NS)
        xshT = np.ascontiguousarray(
            x3[:, sl, :].astype(np.float16).transpose(2, 0, 1)
            .reshape(DI, bt * NS))
        maps.append({
            "x_r": x_r,
            "xshT": xshT,
            "neT": neT,
            "neTs": np.ascontiguousarray(neT[:, sl]),
            "te": te,
            "wp": wpf,
            "bp": bpf,
        })
    return maps


def kernel(x, node_embeddings, time_embeddings, weights_pool, bias_pool,
           ln_gamma=None, ln_beta=None, _trace=False):
    nc = _get_nc()
    in_maps = make_in_maps(np.asarray(x), np.asarray(node_embeddings),
                           np.asarray(time_embeddings),
                           np.asarray(weights_pool), np.asarray(bias_pool))
    res = run_bass_kernel_spmd(nc, in_maps, core_ids=list(range(N_CORES)),
                               trace=_trace)
    NS = NFULL // N_CORES
    shards = [res.results[c]["out_sh"].reshape(B, T, NS, DO)
              for c in range(N_CORES)]
    out = np.concatenate(shards, axis=2)
    if _trace:
        kernel._last_results = res
    return out


# revision 24
# speedup vs baseline: 1.0917x; 1.0917x over previous
"""Trainium2 Bass kernel for nn_DSTGCM (dynamic spatio-temporal graph conv).

Per-core SPMD: node-shard NS=256 of N=2048, all BT=16 (b,t) pairs.
v2: host-side repack for big DMA descriptors (x as fp16 [bt,p,mc,c] tiles,
pre-transposed ne/ne_sh/x_sh); LN stats via ones-vector matmuls on PE;
QK with shared stationary at N=512 (2 bt per matmul, fp32r full rate);
PV and apply in fp16; Pool engine offload for elementwise; x fully
prefetched to SBUF so the main loop issues no DMAs.
"""

import sys
import numpy as np
from contextlib import ExitStack

for _p in ("/opt/trn_rl_repo",):
    if _p not in sys.path:
        sys.path.insert(0, _p)

import concourse.bass as bass
import concourse.bacc as bacc
import concourse.mybir as mybir
import concourse.tile as tile
from concourse import masks
from concourse.bass_utils import run_bass_kernel_spmd

F32 = mybir.dt.float32
F32R = mybir.dt.float32r
F16 = mybir.dt.float16
AF = mybir.ActivationFunctionType
ALU = mybir.AluOpType

N_CORES = 8
B, T, NFULL, DI, DO, D = 8, 2, 2048, 64, 64, 64
BT = B * T
LN_EPS = 1e-12


def r(ap):
    return ap.bitcast(mybir.dt.float32r)


def build_program(N=NFULL, NS=NFULL // N_CORES, bt=BT, num_devices=N_CORES):
    KIO = 2 * DI * DO  # 8192
    NT = N // 128      # 16 m-chunks
    XW = NT * (DI + 1)  # 1040 free per x tile

    nc = bacc.Bacc("TRN2", target_bir_lowering=False, debug=False,
                   num_devices=num_devices)

    # DRAM inputs (host-repacked)
    x_r = nc.dram_tensor("x_r", [bt * 128, XW], F16, kind="ExternalInput").ap()
    xshT = nc.dram_tensor("xshT", [DI, bt * NS], F16, kind="ExternalInput").ap()
    neT = nc.dram_tensor("neT", [D, N], F32R, kind="ExternalInput").ap()
    neTs = nc.dram_tensor("neTs", [D, NS], F32R, kind="ExternalInput").ap()
    te = nc.dram_tensor("te", [bt, D], F32, kind="ExternalInput").ap()
    wp = nc.dram_tensor("wp", [D, KIO], F32R, kind="ExternalInput").ap()
    bp = nc.dram_tensor("bp", [D, DO], F32R, kind="ExternalInput").ap()
    out_d = nc.dram_tensor("out_sh", [bt * NS, DO], F32, kind="ExternalOutput").ap()

    with tile.TileContext(nc) as tc, ExitStack() as ctx:
        cst = ctx.enter_context(tc.tile_pool(name="cst", bufs=1))
        big = ctx.enter_context(tc.tile_pool(name="big", bufs=1))

        ident = cst.tile([128, 128], F32)
        masks.make_identity(nc, ident[:])
        neg64 = cst.tile([128, 1], F32)
        nc.vector.memset(neg64[:], -64.0)
        ones64 = cst.tile([64, 1], F32)
        nc.vector.memset(ones64[:], 1.0)

        # persistent SBUF tensors
        xs = big.tile([128, bt * XW], F16)          # all x, [p, (b, mc, c)]
        xs_v = xs[:].rearrange("p (b w) -> p b w", b=bt)
        neTt = big.tile([64, N], F32R)               # raw ne^T
        neTst = cst.tile([64, NS], F32R)             # raw ne_sh^T
        necT = cst.tile([66, N], F32R)               # [0:64] centered, 64 ones, 65 var
        necTs = cst.tile([66, NS], F32R)
        tecT = cst.tile([64, bt], F32)
        teT = cst.tile([64, bt], F32R)
        preVarT = cst.tile([66, bt], F32R)
        ckT = cst.tile([65, bt], F32R)
        rstdF = cst.tile([16, N], F32)
        rstdS = cst.tile([16, NS], F32)
        rstdT = cst.tile([128, NT * 16], F32)
        crowk = cst.tile([16, NS], F32)
        wpS = big.tile([64, KIO], F32R)
        bpS = cst.tile([64, DO], F32R)
        teS = cst.tile([16, D], F32)
        Wsb = big.tile([128, DO * NS], F16)         # free (o, n)
        W_v = Wsb[:].rearrange("ki (o n) -> ki o n", o=DO)
        xgT = big.tile([128, bt * NS], F16)         # free (b, n)
        xgT_v = xgT[:].rearrange("ki (b n) -> ki b n", b=bt)
        zrow = big.tile([1, bt * NS], F16)          # softmax denominators
        ztt = cst.tile([16, NS], F16)               # Z gathered to partitions
        rzt = cst.tile([16, NS], F32)               # 1/Z
        biasR = cst.tile([16, 8 * DO], F32)

        # ---- prefetch all DMAs (prep tensors first) ----
        nc.sync.dma_start(neTt[:], neT)
        nc.sync.dma_start(neTst[:], neTs)
        nc.sync.dma_start(teS[:bt, :], te)
        nc.sync.dma_start(bpS[:], bp)
        for q in range(4):
            nc.scalar.dma_start(wpS[:, q * 2048:(q + 1) * 2048],
                                wp[:, q * 2048:(q + 1) * 2048])
        nc.sync.dma_start(xgT[0:64, :], xshT)
        for b_ in range(bt):
            nc.gpsimd.dma_start(xs_v[:, b_, :], x_r[b_ * 128:(b_ + 1) * 128, :])

        with tc.tile_pool(name="pp", bufs=4, space="PSUM") as pp, \
             tc.tile_pool(name="wk", bufs=1) as wk:
            # ---- te stats (tiny) ----
            mus = cst.tile([16, 1], F32)
            nc.vector.reduce_sum(mus[:bt], teS[:bt, :], axis=mybir.AxisListType.X)
            mu = cst.tile([16, 1], F32)
            nc.vector.tensor_scalar_mul(mu[:bt], mus[:bt], 1.0 / D)
            tec = cst.tile([16, D], F32)
            nc.vector.tensor_scalar_sub(tec[:bt], teS[:bt, :], mu[:bt])
            sq = cst.tile([16, D], F32)
            kap = cst.tile([16, 1], F32)
            nc.vector.tensor_tensor(out=sq[:bt], in0=tec[:bt], in1=tec[:bt],
                                    op=ALU.mult)
            nc.vector.reduce_sum(kap[:bt], sq[:bt], axis=mybir.AxisListType.X)

            pv = cst.tile([16, 66], F32)
            nc.vector.tensor_scalar_mul(pv[:bt, 0:64], tec[:bt], 2.0 / D)
            nc.vector.tensor_scalar_mul(pv[:bt, 64:65], kap[:bt], 1.0 / D)
            nc.vector.memset(pv[:bt, 65:66], 1.0)
            pps = pp.tile([66, 16], F32, tag="pp")
            nc.tensor.transpose(pps[:, :bt], pv[:bt, :], ident[:bt, :bt])
            nc.scalar.copy(preVarT[:, :bt], pps[:, :bt])

            pc = cst.tile([16, 65], F32)
            nc.vector.tensor_copy(pc[:bt, 0:64], tec[:bt])
            nc.vector.tensor_copy(pc[:bt, 64:65], kap[:bt])
            pps2 = pp.tile([65, 16], F32, tag="pp")
            nc.tensor.transpose(pps2[:, :bt], pc[:bt, :], ident[:bt, :bt])
            nc.scalar.copy(ckT[:, :bt], pps2[:, :bt])

            pps3 = pp.tile([64, 16], F32, tag="pp")
            nc.tensor.transpose(pps3[:, :bt], tec[:bt, :], ident[:bt, :bt])
            nc.scalar.copy(tecT[:, :bt], pps3[:, :bt])
            pps4 = pp.tile([64, 16], F32, tag="pp")
            nc.tensor.transpose(pps4[:, :bt], teS[:bt, :], ident[:bt, :bt])
            nc.scalar.copy(teT[:, :bt], pps4[:, :bt])

            # ---- ne stats via ones-matmuls (full + shard) ----
            def ne_stats(srct, ncols, dst):
                src = srct[:]
                # mean
                mrow = wk.tile([1, ncols], F32, tag="sC" + str(ncols))
                for jj in range(0, ncols, 512):
                    w = min(512, ncols - jj)
                    pm = pp.tile([1, 512], F32, tag="pp")
                    nc.tensor.matmul(pm[:, :w], r(ones64[:]),
                                     r(src[:, jj:jj + w]), start=True, stop=True)
                    nc.vector.tensor_scalar_mul(mrow[:, jj:jj + w], pm[:, :w],
                                                1.0 / D)
                # center: dst[0:64] = src - mean (bcast over partitions)
                mb = wk.tile([64, ncols], F32, tag="sA" + str(ncols))
                nc.sync.dma_start(
                    mb[:], mrow[:].unsqueeze(1).broadcast_to([1, 64, ncols]))
                nc.vector.tensor_tensor(out=dst[0:64, :],
                                        in0=src[:].bitcast(F32),
                                        in1=mb[:], op=ALU.subtract)
                nc.gpsimd.memset(dst[64:65, :].bitcast(F32), 1.0)
                # var = sum(centered^2)/D
                sqc = wk.tile([64, ncols], F32R, tag="sB" + str(ncols))
                nc.scalar.activation(sqc[:], dst[0:64, :].bitcast(F32),
                                     AF.Square)
                vrow = wk.tile([1, ncols], F32R, tag="sC" + str(ncols))
                for jj in range(0, ncols, 512):
                    w = min(512, ncols - jj)
                    pv_ = pp.tile([1, 512], F32, tag="pp")
                    nc.tensor.matmul(pv_[:, :w], r(ones64[:]),
                                     r(sqc[:, jj:jj + w]), start=True, stop=True)
                    nc.vector.tensor_scalar_mul(vrow[:, jj:jj + w],
                                                pv_[:, :w], 1.0 / D)
                nc.sync.dma_start(dst[65:66, :], vrow[:])

            ne_stats(neTt, N, necT)
            ne_stats(neTst, NS, necTs)

            # ---- rstd ----
            def rstd_from(necT_src, ncols, dst):
                t1 = wk.tile([16, ncols], F32, tag="sA" + str(ncols))
                for jj in range(0, ncols, 512):
                    w = min(512, ncols - jj)
                    pvv = pp.tile([16, 512], F32, tag="pp")
                    nc.tensor.matmul(pvv[:bt, :w], r(preVarT[:, :bt]),
                                     r(necT_src[:, jj:jj + w]), start=True,
                                     stop=True)
                    nc.vector.tensor_scalar_add(t1[:bt, jj:jj + w], pvv[:bt, :w],
                                                LN_EPS)
                t2 = wk.tile([16, ncols], F32, tag="sB" + str(ncols))
                nc.vector.reciprocal(t2[:bt], t1[:bt])
                nc.scalar.activation(dst[:bt, :], t2[:bt], AF.Sqrt)

            rstd_from(necT, N, rstdF)
            rstd_from(necTs, NS, rstdS)

            pck = pp.tile([16, NS], F32, tag="pp")
            nc.tensor.matmul(pck[:bt, :], r(ckT[:, :bt]), r(necTs[0:65, :]),
                             start=True, stop=True)
            nc.scalar.copy(crowk[:bt, :], pck[:bt, :])

            for i in range(NT):
                pt = pp.tile([128, 16], F32, tag="pp")
                nc.tensor.transpose(pt[:, :bt], rstdF[:bt, i * 128:(i + 1) * 128],
                                    ident[:bt, :bt])
                nc.scalar.copy(rstdT[:, i * 16:i * 16 + bt], pt[:, :bt])

            # bias
            pb = pp.tile([16, DO], F32, tag="pp")
            nc.tensor.matmul(pb[:bt, :], r(teT[:, :bt]), r(bpS[:, :]),
                             start=True, stop=True)
            bias = cst.tile([16, DO], F32)
            nc.scalar.copy(bias[:bt, :], pb[:bt, :])
            for jj in range(8):
                nc.gpsimd.tensor_copy(biasR[:bt, jj * DO:(jj + 1) * DO],
                                      bias[:bt, :])

        # ---- per-node weights: W[ki, o, n] fp16 ----
        with tc.tile_pool(name="pwp", bufs=4, space="PSUM") as pwp:
            wp_v = wpS[:].rearrange("d (ki o) -> d o ki", o=DO)
            for o in range(DO):
                pw = pwp.tile([128, NS], F32, tag="pw")
                nc.tensor.matmul(pw[:], r(wp_v[:, o, :]), r(neTst[:, :]),
                                 start=True, stop=True)
                if o % 2 == 0:
                    nc.vector.tensor_copy(W_v[:, o, :], pw[:])
                else:
                    nc.scalar.activation(W_v[:, o, :], pw[:], AF.Copy)

        # ---- main loop: 8 bt-pairs x 16 m-chunks ----
        with tc.tile_pool(name="qkp", bufs=4, space="PSUM") as qkp, \
             tc.tile_pool(name="pvp", bufs=2, space="PSUM") as pvp, \
             tc.tile_pool(name="esp", bufs=6) as esp, \
             tc.tile_pool(name="rhp", bufs=2) as rhp:
            for pr in range(bt // 2):
                b0, b1 = 2 * pr, 2 * pr + 1
                prh = rhp.tile([65, 2 * NS], F32, tag="prh")
                pbc = rhp.tile([65, 2 * NS], F32, tag="pbc")
                for h, b_ in enumerate((b0, b1)):
                    sl = slice(h * NS, (h + 1) * NS)
                    nc.vector.tensor_scalar_add(prh[0:64, sl],
                                                necTs[0:64, :].bitcast(F32),
                                                tecT[:, b_:b_ + 1])
                    nc.sync.dma_start(prh[64:65, sl], crowk[b_:b_ + 1, :])
                    nc.sync.dma_start(pbc[:, sl],
                                      rstdS[b_:b_ + 1, :].unsqueeze(1)
                                      .broadcast_to([1, 65, NS]))
                rhs = rhp.tile([65, 2 * NS], F32R, tag="rhs")
                nc.vector.tensor_tensor(out=rhs[:], in0=pbc[:], in1=prh[:],
                                        op=ALU.mult)

                ppv0 = pvp.tile([65, NS], F32, tag="ppv0")
                ppv1 = pvp.tile([65, NS], F32, tag="ppv1")
                ppvs = [ppv0, ppv1]
                for mc in range(NT):
                    pS = qkp.tile([128, 2 * NS], F32, tag="pS")
                    nc.tensor.matmul(pS[:],
                                     r(necT[0:65, mc * 128:(mc + 1) * 128]),
                                     r(rhs[:]), start=True, stop=True)
                    es = esp.tile([128, 2 * NS], F16, tag="es")
                    for h, b_ in enumerate((b0, b1)):
                        sl = slice(h * NS, (h + 1) * NS)
                        nc.scalar.activation(
                            es[:, sl], pS[:, sl], AF.Exp, bias=neg64[:],
                            scale=rstdT[:, mc * 16 + b_:mc * 16 + b_ + 1])
                    for h, b_ in enumerate((b0, b1)):
                        sl = slice(h * NS, (h + 1) * NS)
                        xo = xs_v[:, b_, mc * 65:(mc + 1) * 65]
                        nc.tensor.matmul(ppvs[h][:], xo, es[:, sl],
                                         start=(mc == 0), stop=(mc == NT - 1))

                for h, b_ in enumerate((b0, b1)):
                    nc.vector.tensor_copy(zrow[0:1, b_ * NS:(b_ + 1) * NS],
                                          ppvs[h][64:65, :])
                nc.sync.dma_start(
                    ztt[2 * pr:2 * pr + 2, :],
                    zrow[0:1, 2 * pr * NS:(2 * pr + 2) * NS]
                    .rearrange("one (b n) -> one b n", b=2))
                nc.vector.reciprocal(rzt[:bt, :], ztt[:bt, :])
                for h, b_ in enumerate((b0, b1)):
                    rb = rhp.tile([64, NS], F32, tag="rb")
                    nc.sync.dma_start(rb[:], rzt[b_:b_ + 1, :].unsqueeze(1)
                                      .broadcast_to([1, 64, NS]))
                    nc.vector.tensor_tensor(out=xgT_v[64:128, b_, :],
                                            in0=ppvs[h][0:64, :], in1=rb[:],
                                            op=ALU.mult)

        # ---- apply: per-node matmuls, fp16, batched 8 nodes per PSUM tile ----
        out_v = out_d.rearrange("(b n) o -> b (n o)", b=bt)
        with tc.tile_pool(name="pap", bufs=4, space="PSUM") as pap, \
             tc.tile_pool(name="oap", bufs=3) as oap:
            for nb in range(NS // 8):
                pA = pap.tile([16, 8 * DO], F32, tag="pA")
                for jj in range(8):
                    n_ = nb * 8 + jj
                    nc.tensor.matmul(pA[:bt, jj * DO:(jj + 1) * DO],
                                     xgT_v[:, :, n_], W_v[:, :, n_],
                                     start=True, stop=True)
                ob = oap.tile([16, 8 * DO], F32, tag="ob")
                nc.vector.tensor_tensor(out=ob[:bt, :], in0=pA[:bt, :],
                                        in1=biasR[:bt, :], op=ALU.add)
                nc.sync.dma_start(
                    out_v[:, nb * 8 * DO:(nb + 1) * 8 * DO], ob[:bt, :])

    nc.compile()
    return nc


_NC_CACHE = {}


def _get_nc():
    if "nc" not in _NC_CACHE:
        _NC_CACHE["nc"] = build_program()
    return _NC_CACHE["nc"]


def make_in_maps(x, node_embeddings, time_embeddings, weights_pool, bias_pool,
                 N=NFULL, bt=BT, n_cores=N_CORES):
    NS = N // n_cores
    NT = N // 128
    # x_r: [bt, mc, p, c] -> [p, bt, mc, c] -> rows (b*128+p), 1040 cols
    x4 = np.concatenate(
        [x.reshape(bt, N, DI).astype(np.float16),
         np.ones((bt, N, 1), np.float16)], axis=2)      # (bt, N, 65)
    x_r = np.ascontiguousarray(
        x4.reshape(bt, NT, 128, DI + 1).transpose(0, 2, 1, 3)
        .reshape(bt * 128, NT * (DI + 1)))
    ne = node_embeddings.astype(np.float32)
    neT = np.ascontiguousarray(ne.T)                    # (64, N)
    te = np.ascontiguousarray(time_embeddings.reshape(bt, D).astype(np.float32))
    wpf = np.ascontiguousarray(
        weights_pool.reshape(D, 2 * DI * DO).astype(np.float32))
    bpf = np.ascontiguousarray(bias_pool.astype(np.float32))
    x3 = x.reshape(bt, N, DI)
    maps = []
    for c in range(n_cores):
        sl = slice(c * NS, (c + 1) * NS)
        xshT = np.ascontiguousarray(
            x3[:, sl, :].astype(np.float16).transpose(2, 0, 1)
            .reshape(DI, bt * NS))
        maps.append({
            "x_r": x_r,
            "xshT": xshT,
            "neT": neT,
            "neTs": np.ascontiguousarray(neT[:, sl]),
            "te": te,
            "wp": wpf,
            "bp": bpf,
        })
    return maps


def kernel(x, node_embeddings, time_embeddings, weights_pool, bias_pool,
           ln_gamma=None, ln_beta=None, _trace=False):
    nc = _get_nc()
    in_maps = make_in_maps(np.asarray(x), np.asarray(node_embeddings),
                           np.asarray(time_embeddings),
                           np.asarray(weights_pool), np.asarray(bias_pool))
    res = run_bass_kernel_spmd(nc, in_maps, core_ids=list(range(N_CORES)),
                               trace=_trace)
    NS = NFULL // N_CORES
    shards = [res.results[c]["out_sh"].reshape(B, T, NS, DO)
              for c in range(N_CORES)]
    out = np.concatenate(shards, axis=2)
    if _trace:
        kernel._last_results = res
    return out


# revision 25
# speedup vs baseline: 1.0996x; 1.0072x over previous
"""Trainium2 Bass kernel for nn_DSTGCM (dynamic spatio-temporal graph conv).

Per-core SPMD: node-shard NS=256 of N=2048, all BT=16 (b,t) pairs.
v2: host-side repack for big DMA descriptors (x as fp16 [bt,p,mc,c] tiles,
pre-transposed ne/ne_sh/x_sh); LN stats via ones-vector matmuls on PE;
QK with shared stationary at N=512 (2 bt per matmul, fp32r full rate);
PV and apply in fp16; Pool engine offload for elementwise; x fully
prefetched to SBUF so the main loop issues no DMAs.
"""

import sys
import numpy as np
from contextlib import ExitStack

for _p in ("/opt/trn_rl_repo",):
    if _p not in sys.path:
        sys.path.insert(0, _p)

import concourse.bass as bass
import concourse.bacc as bacc
import concourse.mybir as mybir
import concourse.tile as tile
from concourse import masks
from concourse.bass_utils import run_bass_kernel_spmd

F32 = mybir.dt.float32
F32R = mybir.dt.float32r
F16 = mybir.dt.float16
AF = mybir.ActivationFunctionType
ALU = mybir.AluOpType

N_CORES = 8
B, T, NFULL, DI, DO, D = 8, 2, 2048, 64, 64, 64
BT = B * T
LN_EPS = 1e-12


def r(ap):
    return ap.bitcast(mybir.dt.float32r)


def build_program(N=NFULL, NS=NFULL // N_CORES, bt=BT, num_devices=N_CORES):
    KIO = 2 * DI * DO  # 8192
    NT = N // 128      # 16 m-chunks
    XW = NT * (DI + 1)  # 1040 free per x tile

    nc = bacc.Bacc("TRN2", target_bir_lowering=False, debug=False,
                   num_devices=num_devices)

    # DRAM inputs (host-repacked)
    x_r = nc.dram_tensor("x_r", [bt * 128, XW], F16, kind="ExternalInput").ap()
    xshT = nc.dram_tensor("xshT", [DI, bt * NS], F16, kind="ExternalInput").ap()
    neT = nc.dram_tensor("neT", [D, N], F32R, kind="ExternalInput").ap()
    neTs = nc.dram_tensor("neTs", [D, NS], F32R, kind="ExternalInput").ap()
    te = nc.dram_tensor("te", [bt, D], F32, kind="ExternalInput").ap()
    wp = nc.dram_tensor("wp", [D, KIO], F32R, kind="ExternalInput").ap()
    bp = nc.dram_tensor("bp", [D, DO], F32R, kind="ExternalInput").ap()
    out_d = nc.dram_tensor("out_sh", [bt * NS, DO], F32, kind="ExternalOutput").ap()

    with tile.TileContext(nc) as tc, ExitStack() as ctx:
        cst = ctx.enter_context(tc.tile_pool(name="cst", bufs=1))
        big = ctx.enter_context(tc.tile_pool(name="big", bufs=1))

        ident = cst.tile([128, 128], F32)
        masks.make_identity(nc, ident[:])
        neg64 = cst.tile([128, 1], F32)
        nc.vector.memset(neg64[:], -64.0)
        ones64 = cst.tile([64, 1], F32)
        nc.vector.memset(ones64[:], 1.0)

        # persistent SBUF tensors
        xs = big.tile([128, bt * XW], F16)          # all x, [p, (b, mc, c)]
        xs_v = xs[:].rearrange("p (b w) -> p b w", b=bt)
        neTt = big.tile([64, N], F32R)               # raw ne^T
        neTst = cst.tile([64, NS], F32R)             # raw ne_sh^T
        necT = cst.tile([66, N], F32R)               # [0:64] centered, 64 ones, 65 var
        necTs = cst.tile([66, NS], F32R)
        tecT = cst.tile([64, bt], F32)
        teT = cst.tile([64, bt], F32R)
        preVarT = cst.tile([66, bt], F32R)
        ckT = cst.tile([65, bt], F32R)
        rstdF = cst.tile([16, N], F32)
        rstdS = cst.tile([16, NS], F32)
        rstdT = cst.tile([128, NT * 16], F32)
        crowk = cst.tile([16, NS], F32)
        wpS = big.tile([64, KIO], F32R)
        bpS = cst.tile([64, DO], F32R)
        teS = cst.tile([16, D], F32)
        Wsb = big.tile([128, DO * NS], F16)         # free (o, n)
        W_v = Wsb[:].rearrange("ki (o n) -> ki o n", o=DO)
        xgT = big.tile([128, bt * NS], F16)         # free (b, n)
        xgT_v = xgT[:].rearrange("ki (b n) -> ki b n", b=bt)
        zrow = big.tile([1, bt * NS], F16)          # softmax denominators
        ztt = cst.tile([16, NS], F16)               # Z gathered to partitions
        rzt = cst.tile([16, NS], F32)               # 1/Z
        biasR = cst.tile([16, 8 * DO], F32)

        # ---- prefetch all DMAs (prep tensors first) ----
        nc.sync.dma_start(neTt[:], neT)
        nc.sync.dma_start(neTst[:], neTs)
        nc.sync.dma_start(teS[:bt, :], te)
        nc.sync.dma_start(bpS[:], bp)
        for q in range(4):
            nc.scalar.dma_start(wpS[:, q * 2048:(q + 1) * 2048],
                                wp[:, q * 2048:(q + 1) * 2048])
        nc.sync.dma_start(xgT[0:64, :], xshT)
        for b_ in range(bt):
            nc.gpsimd.dma_start(xs_v[:, b_, :], x_r[b_ * 128:(b_ + 1) * 128, :])

        with tc.tile_pool(name="pp", bufs=4, space="PSUM") as pp, \
             tc.tile_pool(name="wk", bufs=1) as wk:
            # ---- te stats (tiny) ----
            mus = cst.tile([16, 1], F32)
            nc.vector.reduce_sum(mus[:bt], teS[:bt, :], axis=mybir.AxisListType.X)
            mu = cst.tile([16, 1], F32)
            nc.vector.tensor_scalar_mul(mu[:bt], mus[:bt], 1.0 / D)
            tec = cst.tile([16, D], F32)
            nc.vector.tensor_scalar_sub(tec[:bt], teS[:bt, :], mu[:bt])
            sq = cst.tile([16, D], F32)
            kap = cst.tile([16, 1], F32)
            nc.vector.tensor_tensor(out=sq[:bt], in0=tec[:bt], in1=tec[:bt],
                                    op=ALU.mult)
            nc.vector.reduce_sum(kap[:bt], sq[:bt], axis=mybir.AxisListType.X)

            pv = cst.tile([16, 66], F32)
            nc.vector.tensor_scalar_mul(pv[:bt, 0:64], tec[:bt], 2.0 / D)
            nc.vector.tensor_scalar_mul(pv[:bt, 64:65], kap[:bt], 1.0 / D)
            nc.vector.memset(pv[:bt, 65:66], 1.0)
            pps = pp.tile([66, 16], F32, tag="pp")
            nc.tensor.transpose(pps[:, :bt], pv[:bt, :], ident[:bt, :bt])
            nc.scalar.copy(preVarT[:, :bt], pps[:, :bt])

            pc = cst.tile([16, 65], F32)
            nc.vector.tensor_copy(pc[:bt, 0:64], tec[:bt])
            nc.vector.tensor_copy(pc[:bt, 64:65], kap[:bt])
            pps2 = pp.tile([65, 16], F32, tag="pp")
            nc.tensor.transpose(pps2[:, :bt], pc[:bt, :], ident[:bt, :bt])
            nc.scalar.copy(ckT[:, :bt], pps2[:, :bt])

            pps3 = pp.tile([64, 16], F32, tag="pp")
            nc.tensor.transpose(pps3[:, :bt], tec[:bt, :], ident[:bt, :bt])
            nc.scalar.copy(tecT[:, :bt], pps3[:, :bt])
            pps4 = pp.tile([64, 16], F32, tag="pp")
            nc.tensor.transpose(pps4[:, :bt], teS[:bt, :], ident[:bt, :bt])
            nc.scalar.copy(teT[:, :bt], pps4[:, :bt])

            # ---- ne stats via ones-matmuls (full + shard) ----
            def ne_stats(srct, ncols, dst):
                src = srct[:]
                # mean
                mrow = wk.tile([1, ncols], F32, tag="sC" + str(ncols))
                for jj in range(0, ncols, 512):
                    w = min(512, ncols - jj)
                    pm = pp.tile([1, 512], F32, tag="pp")
                    nc.tensor.matmul(pm[:, :w], r(ones64[:]),
                                     r(src[:, jj:jj + w]), start=True, stop=True)
                    nc.vector.tensor_scalar_mul(mrow[:, jj:jj + w], pm[:, :w],
                                                1.0 / D)
                # center: dst[0:64] = src - mean (bcast over partitions)
                mb = wk.tile([64, ncols], F32, tag="sA" + str(ncols))
                nc.sync.dma_start(
                    mb[:], mrow[:].unsqueeze(1).broadcast_to([1, 64, ncols]))
                nc.vector.tensor_tensor(out=dst[0:64, :],
                                        in0=src[:].bitcast(F32),
                                        in1=mb[:], op=ALU.subtract)
                nc.gpsimd.memset(dst[64:65, :].bitcast(F32), 1.0)
                # var = sum(centered^2)/D
                sqc = wk.tile([64, ncols], F32R, tag="sB" + str(ncols))
                nc.scalar.activation(sqc[:], dst[0:64, :].bitcast(F32),
                                     AF.Square)
                vrow = wk.tile([1, ncols], F32R, tag="sC" + str(ncols))
                for jj in range(0, ncols, 512):
                    w = min(512, ncols - jj)
                    pv_ = pp.tile([1, 512], F32, tag="pp")
                    nc.tensor.matmul(pv_[:, :w], r(ones64[:]),
                                     r(sqc[:, jj:jj + w]), start=True, stop=True)
                    nc.vector.tensor_scalar_mul(vrow[:, jj:jj + w],
                                                pv_[:, :w], 1.0 / D)
                nc.sync.dma_start(dst[65:66, :], vrow[:])

            ne_stats(neTt, N, necT)
            ne_stats(neTst, NS, necTs)

            # ---- rstd ----
            def rstd_from(necT_src, ncols, dst):
                t1 = wk.tile([16, ncols], F32, tag="sA" + str(ncols))
                for jj in range(0, ncols, 512):
                    w = min(512, ncols - jj)
                    pvv = pp.tile([16, 512], F32, tag="pp")
                    nc.tensor.matmul(pvv[:bt, :w], r(preVarT[:, :bt]),
                                     r(necT_src[:, jj:jj + w]), start=True,
                                     stop=True)
                    nc.vector.tensor_scalar_add(t1[:bt, jj:jj + w], pvv[:bt, :w],
                                                LN_EPS)
                t2 = wk.tile([16, ncols], F32, tag="sB" + str(ncols))
                nc.vector.reciprocal(t2[:bt], t1[:bt])
                nc.scalar.activation(dst[:bt, :], t2[:bt], AF.Sqrt)

            rstd_from(necT, N, rstdF)
            rstd_from(necTs, NS, rstdS)

            pck = pp.tile([16, NS], F32, tag="pp")
            nc.tensor.matmul(pck[:bt, :], r(ckT[:, :bt]), r(necTs[0:65, :]),
                             start=True, stop=True)
            nc.scalar.copy(crowk[:bt, :], pck[:bt, :])

            for i in range(NT):
                pt = pp.tile([128, 16], F32, tag="pp")
                nc.tensor.transpose(pt[:, :bt], rstdF[:bt, i * 128:(i + 1) * 128],
                                    ident[:bt, :bt])
                nc.scalar.copy(rstdT[:, i * 16:i * 16 + bt], pt[:, :bt])

            # bias
            pb = pp.tile([16, DO], F32, tag="pp")
            nc.tensor.matmul(pb[:bt, :], r(teT[:, :bt]), r(bpS[:, :]),
                             start=True, stop=True)
            bias = cst.tile([16, DO], F32)
            nc.scalar.copy(bias[:bt, :], pb[:bt, :])
            for jj in range(8):
                nc.gpsimd.tensor_copy(biasR[:bt, jj * DO:(jj + 1) * DO],
                                      bias[:bt, :])

        # ---- per-node weights: W[ki, o, n] fp16 ----
        with tc.tile_pool(name="pwp", bufs=4, space="PSUM") as pwp:
            wp_v = wpS[:].rearrange("d (ki o) -> d o ki", o=DO)
            for o in range(DO):
                pw = pwp.tile([128, NS], F32, tag="pw")
                nc.tensor.matmul(pw[:], r(wp_v[:, o, :]), r(neTst[:, :]),
                                 start=True, stop=True)
                if o % 2 == 0:
                    nc.vector.tensor_copy(W_v[:, o, :], pw[:])
                else:
                    nc.scalar.activation(W_v[:, o, :], pw[:], AF.Copy)

        # ---- main loop: 8 bt-pairs x 16 m-chunks ----
        with tc.tile_pool(name="qkp", bufs=4, space="PSUM") as qkp, \
             tc.tile_pool(name="pvp", bufs=2, space="PSUM") as pvp, \
             tc.tile_pool(name="esp", bufs=6) as esp, \
             tc.tile_pool(name="rhp", bufs=2) as rhp:
            for pr in range(bt // 2):
                b0, b1 = 2 * pr, 2 * pr + 1
                prh = rhp.tile([65, 2 * NS], F32, tag="prh")
                pbc = rhp.tile([65, 2 * NS], F32, tag="pbc")
                for h, b_ in enumerate((b0, b1)):
                    sl = slice(h * NS, (h + 1) * NS)
                    nc.vector.tensor_scalar_add(prh[0:64, sl],
                                                necTs[0:64, :].bitcast(F32),
                                                tecT[:, b_:b_ + 1])
                    nc.sync.dma_start(prh[64:65, sl], crowk[b_:b_ + 1, :])
                    nc.sync.dma_start(pbc[:, sl],
                                      rstdS[b_:b_ + 1, :].unsqueeze(1)
                                      .broadcast_to([1, 65, NS]))
                rhs = rhp.tile([65, 2 * NS], F32R, tag="rhs")
                nc.vector.tensor_tensor(out=rhs[:], in0=pbc[:], in1=prh[:],
                                        op=ALU.mult)

                ppv0 = pvp.tile([65, NS], F32, tag="ppv0")
                ppv1 = pvp.tile([65, NS], F32, tag="ppv1")
                ppvs = [ppv0, ppv1]
                for mc in range(NT):
                    pS = qkp.tile([128, 2 * NS], F32, tag="pS")
                    nc.tensor.matmul(pS[:],
                                     r(necT[0:65, mc * 128:(mc + 1) * 128]),
                                     r(rhs[:]), start=True, stop=True)
                    es = esp.tile([128, 2 * NS], F16, tag="es")
                    for h, b_ in enumerate((b0, b1)):
                        sl = slice(h * NS, (h + 1) * NS)
                        nc.scalar.activation(
                            es[:, sl], pS[:, sl], AF.Exp, bias=neg64[:],
                            scale=rstdT[:, mc * 16 + b_:mc * 16 + b_ + 1])
                    for h, b_ in enumerate((b0, b1)):
                        sl = slice(h * NS, (h + 1) * NS)
                        xo = xs_v[:, b_, mc * 65:(mc + 1) * 65]
                        nc.tensor.matmul(ppvs[h][:], xo, es[:, sl],
                                         start=(mc == 0), stop=(mc == NT - 1))

                for h, b_ in enumerate((b0, b1)):
                    nc.vector.tensor_copy(zrow[0:1, b_ * NS:(b_ + 1) * NS],
                                          ppvs[h][64:65, :])
                nc.sync.dma_start(
                    ztt[2 * pr:2 * pr + 2, :],
                    zrow[0:1, 2 * pr * NS:(2 * pr + 2) * NS]
                    .rearrange("one (b n) -> one b n", b=2))
                nc.vector.reciprocal(rzt[:bt, :], ztt[:bt, :])
                for h, b_ in enumerate((b0, b1)):
                    rb = rhp.tile([64, NS], F32, tag="rb")
                    nc.sync.dma_start(rb[:], rzt[b_:b_ + 1, :].unsqueeze(1)
                                      .broadcast_to([1, 64, NS]))
                    nc.vector.tensor_tensor(out=xgT_v[64:128, b_, :],
                                            in0=ppvs[h][0:64, :], in1=rb[:],
                                            op=ALU.mult)

        # ---- apply: per-node matmuls, fp16, batched 8 nodes per PSUM tile ----
        out_v = out_d.rearrange("(b n) o -> b (n o)", b=bt)
        with tc.tile_pool(name="pap", bufs=4, space="PSUM") as pap, \
             tc.tile_pool(name="oap", bufs=3) as oap:
            for nb in range(NS // 8):
                pA = pap.tile([16, 8 * DO], F32, tag="pA")
                for jj in range(8):
                    n_ = nb * 8 + jj
                    nc.tensor.matmul(pA[:bt, jj * DO:(jj + 1) * DO],
                                     xgT_v[:, :, n_], W_v[:, :, n_],
                                     start=True, stop=True)
                ob = oap.tile([16, 8 * DO], F32, tag="ob")
                nc.vector.tensor_tensor(out=ob[:bt, :], in0=pA[:bt, :],
                                        in1=biasR[:bt, :], op=ALU.add)
                nc.scalar.dma_start(
                    out_v[:, nb * 8 * DO:(nb + 1) * 8 * DO], ob[:bt, :])

    nc.compile()
    return nc


_NC_CACHE = {}


def _get_nc():
    if "nc" not in _NC_CACHE:
        _NC_CACHE["nc"] = build_program()
    return _NC_CACHE["nc"]


def make_in_maps(x, node_embeddings, time_embeddings, weights_pool, bias_pool,
                 N=NFULL, bt=BT, n_cores=N_CORES):
    NS = N // n_cores
    NT = N // 128
    # x_r: [bt, mc, p, c] -> [p, bt, mc, c] -> rows (b*128+p), 1040 cols
    x4 = np.concatenate(
        [x.reshape(bt, N, DI).astype(np.float16),
         np.ones((bt, N, 1), np.float16)], axis=2)      # (bt, N, 65)
    x_r = np.ascontiguousarray(
        x4.reshape(bt, NT, 128, DI + 1).transpose(0, 2, 1, 3)
        .reshape(bt * 128, NT * (DI + 1)))
    ne = node_embeddings.astype(np.float32)
    neT = np.ascontiguousarray(ne.T)                    # (64, N)
    te = np.ascontiguousarray(time_embeddings.reshape(bt, D).astype(np.float32))
    wpf = np.ascontiguousarray(
        weights_pool.reshape(D, 2 * DI * DO).astype(np.float32))
    bpf = np.ascontiguousarray(bias_pool.astype(np.float32))
    x3 = x.reshape(bt, N, DI)
    maps = []
    for c in range(n_cores):
        sl = slice(c * NS, (c + 1) * NS)
        xshT = np.ascontiguousarray(
            x3[:, sl, :].astype(np.float16).transpose(2, 0, 1)
            .reshape(DI, bt * NS))
        maps.append({
            "x_r": x_r,
            "xshT": xshT,
            "neT": neT,
            "neTs": np.ascontiguousarray(neT[:, sl]),
            "te": te,
            "wp": wpf,
            "bp": bpf,
        })
    return maps


def kernel(x, node_embeddings, time_embeddings, weights_pool, bias_pool,
           ln_gamma=None, ln_beta=None, _trace=False):
    nc = _get_nc()
    in_maps = make_in_maps(np.asarray(x), np.asarray(node_embeddings),
                           np.asarray(time_embeddings),
                           np.asarray(weights_pool), np.asarray(bias_pool))
    res = run_bass_kernel_spmd(nc, in_maps, core_ids=list(range(N_CORES)),
                               trace=_trace)
    NS = NFULL // N_CORES
    shards = [res.results[c]["out_sh"].reshape(B, T, NS, DO)
              for c in range(N_CORES)]
    out = np.concatenate(shards, axis=2)
    if _trace:
        kernel._last_results = res
    return out


# revision 26
# speedup vs baseline: 1.1132x; 1.0124x over previous
"""Trainium2 Bass kernel for nn_DSTGCM (dynamic spatio-temporal graph conv).

Per-core SPMD: node-shard NS=256 of N=2048, all BT=16 (b,t) pairs.
v2: host-side repack for big DMA descriptors (x as fp16 [bt,p,mc,c] tiles,
pre-transposed ne/ne_sh/x_sh); LN stats via ones-vector matmuls on PE;
QK with shared stationary at N=512 (2 bt per matmul, fp32r full rate);
PV and apply in fp16; Pool engine offload for elementwise; x fully
prefetched to SBUF so the main loop issues no DMAs.
"""

import sys
import numpy as np
from contextlib import ExitStack

for _p in ("/opt/trn_rl_repo",):
    if _p not in sys.path:
        sys.path.insert(0, _p)

import concourse.bass as bass
import concourse.bacc as bacc
import concourse.mybir as mybir
import concourse.tile as tile
from concourse import masks
from concourse.bass_utils import run_bass_kernel_spmd

F32 = mybir.dt.float32
F32R = mybir.dt.float32r
F16 = mybir.dt.float16
AF = mybir.ActivationFunctionType
ALU = mybir.AluOpType

N_CORES = 8
B, T, NFULL, DI, DO, D = 8, 2, 2048, 64, 64, 64
BT = B * T
LN_EPS = 1e-12


def r(ap):
    return ap.bitcast(mybir.dt.float32r)


def build_program(N=NFULL, NS=NFULL // N_CORES, bt=BT, num_devices=N_CORES):
    KIO = 2 * DI * DO  # 8192
    NT = N // 128      # 16 m-chunks
    XW = NT * (DI + 1)  # 1040 free per x tile

    nc = bacc.Bacc("TRN2", target_bir_lowering=False, debug=False,
                   num_devices=num_devices)

    # DRAM inputs (host-repacked)
    x_r = nc.dram_tensor("x_r", [bt * 128, XW], F16, kind="ExternalInput").ap()
    xshT = nc.dram_tensor("xshT", [DI, bt * NS], F16, kind="ExternalInput").ap()
    neT = nc.dram_tensor("neT", [D, N], F32R, kind="ExternalInput").ap()
    neTs = nc.dram_tensor("neTs", [D, NS], F32R, kind="ExternalInput").ap()
    te = nc.dram_tensor("te", [bt, D], F32, kind="ExternalInput").ap()
    wp = nc.dram_tensor("wp", [D, KIO], F32R, kind="ExternalInput").ap()
    bp = nc.dram_tensor("bp", [D, DO], F32R, kind="ExternalInput").ap()
    out_d = nc.dram_tensor("out_sh", [bt * NS, DO], F32, kind="ExternalOutput").ap()

    with tile.TileContext(nc) as tc, ExitStack() as ctx:
        cst = ctx.enter_context(tc.tile_pool(name="cst", bufs=1))
        big = ctx.enter_context(tc.tile_pool(name="big", bufs=1))

        ident = cst.tile([128, 128], F32)
        masks.make_identity(nc, ident[:])
        neg64 = cst.tile([128, 1], F32)
        nc.vector.memset(neg64[:], -64.0)
        ones64 = cst.tile([64, 1], F32)
        nc.vector.memset(ones64[:], 1.0)

        # persistent SBUF tensors
        xs = big.tile([128, bt * XW], F16)          # all x, [p, (b, mc, c)]
        xs_v = xs[:].rearrange("p (b w) -> p b w", b=bt)
        neTt = big.tile([64, N], F32R)               # raw ne^T
        neTst = cst.tile([64, NS], F32R)             # raw ne_sh^T
        necT = cst.tile([66, N], F32R)               # [0:64] centered, 64 ones, 65 var
        necTs = cst.tile([66, NS], F32R)
        tecT = cst.tile([64, bt], F32)
        teT = cst.tile([64, bt], F32R)
        preVarT = cst.tile([66, bt], F32R)
        ckT = cst.tile([65, bt], F32R)
        rstdF = cst.tile([16, N], F32)
        rstdS = cst.tile([16, NS], F32)
        rstdT = cst.tile([128, NT * 16], F32)
        crowk = cst.tile([16, NS], F32)
        wpS = big.tile([64, KIO], F32R)
        bpS = cst.tile([64, DO], F32R)
        teS = cst.tile([16, D], F32)
        Wsb = big.tile([128, DO * NS], F16)         # free (o, n)
        W_v = Wsb[:].rearrange("ki (o n) -> ki o n", o=DO)
        xgT = big.tile([128, bt * NS], F16)         # free (b, n)
        xgT_v = xgT[:].rearrange("ki (b n) -> ki b n", b=bt)
        zrow = big.tile([1, bt * NS], F16)          # softmax denominators
        ztt = cst.tile([16, NS], F16)               # Z gathered to partitions
        rzt = cst.tile([16, NS], F32)               # 1/Z
        biasR = cst.tile([16, 8 * DO], F32)

        # ---- prefetch all DMAs (prep tensors first) ----
        nc.sync.dma_start(neTt[:], neT)
        nc.sync.dma_start(neTst[:], neTs)
        nc.sync.dma_start(teS[:bt, :], te)
        nc.sync.dma_start(bpS[:], bp)
        for q in range(4):
            nc.scalar.dma_start(wpS[:, q * 2048:(q + 1) * 2048],
                                wp[:, q * 2048:(q + 1) * 2048])
        nc.sync.dma_start(xgT[0:64, :], xshT)
        for b_ in range(bt):
            nc.gpsimd.dma_start(xs_v[:, b_, :], x_r[b_ * 128:(b_ + 1) * 128, :])

        with tc.tile_pool(name="pp", bufs=4, space="PSUM") as pp, \
             tc.tile_pool(name="wk", bufs=1) as wk:
            # ---- te stats (tiny) ----
            mus = cst.tile([16, 1], F32)
            nc.vector.reduce_sum(mus[:bt], teS[:bt, :], axis=mybir.AxisListType.X)
            mu = cst.tile([16, 1], F32)
            nc.vector.tensor_scalar_mul(mu[:bt], mus[:bt], 1.0 / D)
            tec = cst.tile([16, D], F32)
            nc.vector.tensor_scalar_sub(tec[:bt], teS[:bt, :], mu[:bt])
            sq = cst.tile([16, D], F32)
            kap = cst.tile([16, 1], F32)
            nc.vector.tensor_tensor(out=sq[:bt], in0=tec[:bt], in1=tec[:bt],
                                    op=ALU.mult)
            nc.vector.reduce_sum(kap[:bt], sq[:bt], axis=mybir.AxisListType.X)

            pv = cst.tile([16, 66], F32)
            nc.vector.tensor_scalar_mul(pv[:bt, 0:64], tec[:bt], 2.0 / D)
            nc.vector.tensor_scalar_mul(pv[:bt, 64:65], kap[:bt], 1.0 / D)
            nc.vector.memset(pv[:bt, 65:66], 1.0)
            pps = pp.tile([66, 16], F32, tag="pp")
            nc.tensor.transpose(pps[:, :bt], pv[:bt, :], ident[:bt, :bt])
            nc.scalar.copy(preVarT[:, :bt], pps[:, :bt])

            pc = cst.tile([16, 65], F32)
            nc.vector.tensor_copy(pc[:bt, 0:64], tec[:bt])
            nc.vector.tensor_copy(pc[:bt, 64:65], kap[:bt])
            pps2 = pp.tile([65, 16], F32, tag="pp")
            nc.tensor.transpose(pps2[:, :bt], pc[:bt, :], ident[:bt, :bt])
            nc.scalar.copy(ckT[:, :bt], pps2[:, :bt])

            pps3 = pp.tile([64, 16], F32, tag="pp")
            nc.tensor.transpose(pps3[:, :bt], tec[:bt, :], ident[:bt, :bt])
            nc.scalar.copy(tecT[:, :bt], pps3[:, :bt])
            pps4 = pp.tile([64, 16], F32, tag="pp")
            nc.tensor.transpose(pps4[:, :bt], teS[:bt, :], ident[:bt, :bt])
            nc.scalar.copy(teT[:, :bt], pps4[:, :bt])

            # ---- ne stats via ones-matmuls (full + shard) ----
            def ne_stats(srct, ncols, dst):
                src = srct[:]
                # mean
                mrow = wk.tile([1, ncols], F32, tag="sC" + str(ncols))
                for jj in range(0, ncols, 512):
                    w = min(512, ncols - jj)
                    pm = pp.tile([1, 512], F32, tag="pp")
                    nc.tensor.matmul(pm[:, :w], r(ones64[:]),
                                     r(src[:, jj:jj + w]), start=True, stop=True)
                    nc.vector.tensor_scalar_mul(mrow[:, jj:jj + w], pm[:, :w],
                                                1.0 / D)
                # center: dst[0:64] = src - mean (bcast over partitions)
                mb = wk.tile([64, ncols], F32, tag="sA" + str(ncols))
                nc.sync.dma_start(
                    mb[:], mrow[:].unsqueeze(1).broadcast_to([1, 64, ncols]))
                nc.vector.tensor_tensor(out=dst[0:64, :],
                                        in0=src[:].bitcast(F32),
                                        in1=mb[:], op=ALU.subtract)
                nc.gpsimd.memset(dst[64:65, :].bitcast(F32), 1.0)
                # var = sum(centered^2)/D
                sqc = wk.tile([64, ncols], F32R, tag="sB" + str(ncols))
                nc.scalar.activation(sqc[:], dst[0:64, :].bitcast(F32),
                                     AF.Square)
                vrow = wk.tile([1, ncols], F32R, tag="sC" + str(ncols))
                for jj in range(0, ncols, 512):
                    w = min(512, ncols - jj)
                    pv_ = pp.tile([1, 512], F32, tag="pp")
                    nc.tensor.matmul(pv_[:, :w], r(ones64[:]),
                                     r(sqc[:, jj:jj + w]), start=True, stop=True)
                    nc.vector.tensor_scalar_mul(vrow[:, jj:jj + w],
                                                pv_[:, :w], 1.0 / D)
                nc.sync.dma_start(dst[65:66, :], vrow[:])

            ne_stats(neTt, N, necT)
            ne_stats(neTst, NS, necTs)

            # ---- rstd ----
            def rstd_from(necT_src, ncols, dst):
                t1 = wk.tile([16, ncols], F32, tag="sA" + str(ncols))
                for jj in range(0, ncols, 512):
                    w = min(512, ncols - jj)
                    pvv = pp.tile([16, 512], F32, tag="pp")
                    nc.tensor.matmul(pvv[:bt, :w], r(preVarT[:, :bt]),
                                     r(necT_src[:, jj:jj + w]), start=True,
                                     stop=True)
                    nc.vector.tensor_scalar_add(t1[:bt, jj:jj + w], pvv[:bt, :w],
                                                LN_EPS)
                t2 = wk.tile([16, ncols], F32, tag="sB" + str(ncols))
                nc.vector.reciprocal(t2[:bt], t1[:bt])
                nc.scalar.activation(dst[:bt, :], t2[:bt], AF.Sqrt)

            rstd_from(necT, N, rstdF)
            rstd_from(necTs, NS, rstdS)

            pck = pp.tile([16, NS], F32, tag="pp")
            nc.tensor.matmul(pck[:bt, :], r(ckT[:, :bt]), r(necTs[0:65, :]),
                             start=True, stop=True)
            nc.scalar.copy(crowk[:bt, :], pck[:bt, :])

            for i in range(NT):
                pt = pp.tile([128, 16], F32, tag="pp")
                nc.tensor.transpose(pt[:, :bt], rstdF[:bt, i * 128:(i + 1) * 128],
                                    ident[:bt, :bt])
                nc.scalar.copy(rstdT[:, i * 16:i * 16 + bt], pt[:, :bt])

            # bias
            pb = pp.tile([16, DO], F32, tag="pp")
            nc.tensor.matmul(pb[:bt, :], r(teT[:, :bt]), r(bpS[:, :]),
                             start=True, stop=True)
            bias = cst.tile([16, DO], F32)
            nc.scalar.copy(bias[:bt, :], pb[:bt, :])
            for jj in range(8):
                nc.gpsimd.tensor_copy(biasR[:bt, jj * DO:(jj + 1) * DO],
                                      bias[:bt, :])

        # ---- per-node weights: W[ki, o, n] fp16 ----
        with tc.tile_pool(name="pwp", bufs=4, space="PSUM") as pwp:
            wp_v = wpS[:].rearrange("d (ki o) -> d o ki", o=DO)
            for o in range(DO):
                pw = pwp.tile([128, NS], F32, tag="pw")
                nc.tensor.matmul(pw[:], r(wp_v[:, o, :]), r(neTst[:, :]),
                                 start=True, stop=True)
                if o % 2 == 0:
                    nc.vector.tensor_copy(W_v[:, o, :], pw[:])
                else:
                    nc.scalar.activation(W_v[:, o, :], pw[:], AF.Copy)

        # ---- main loop: 8 bt-pairs x 16 m-chunks ----
        with tc.tile_pool(name="qkp", bufs=4, space="PSUM") as qkp, \
             tc.tile_pool(name="pvp", bufs=2, space="PSUM") as pvp, \
             tc.tile_pool(name="esp", bufs=6) as esp, \
             tc.tile_pool(name="rhp", bufs=2) as rhp:
            for pr in range(bt // 2):
                b0, b1 = 2 * pr, 2 * pr + 1
                prh = rhp.tile([65, 2 * NS], F32, tag="prh")
                pbc = rhp.tile([65, 2 * NS], F32, tag="pbc")
                for h, b_ in enumerate((b0, b1)):
                    sl = slice(h * NS, (h + 1) * NS)
                    nc.vector.tensor_scalar_add(prh[0:64, sl],
                                                necTs[0:64, :].bitcast(F32),
                                                tecT[:, b_:b_ + 1])
                    nc.sync.dma_start(prh[64:65, sl], crowk[b_:b_ + 1, :])
                    nc.sync.dma_start(pbc[:, sl],
                                      rstdS[b_:b_ + 1, :].unsqueeze(1)
                                      .broadcast_to([1, 65, NS]))
                rhs = rhp.tile([65, 2 * NS], F32R, tag="rhs")
                nc.vector.tensor_tensor(out=rhs[:], in0=pbc[:], in1=prh[:],
                                        op=ALU.mult)

                ppv0 = pvp.tile([65, NS], F32, tag="ppv0")
                ppv1 = pvp.tile([65, NS], F32, tag="ppv1")
                ppvs = [ppv0, ppv1]
                for mc in range(NT):
                    pS = qkp.tile([128, 2 * NS], F32, tag="pS")
                    nc.tensor.matmul(pS[:],
                                     r(necT[0:65, mc * 128:(mc + 1) * 128]),
                                     r(rhs[:]), start=True, stop=True)
                    es = esp.tile([128, 2 * NS], F16, tag="es")
                    for h, b_ in enumerate((b0, b1)):
                        sl = slice(h * NS, (h + 1) * NS)
                        nc.scalar.activation(
                            es[:, sl], pS[:, sl], AF.Exp, bias=neg64[:],
                            scale=rstdT[:, mc * 16 + b_:mc * 16 + b_ + 1])
                    for h, b_ in enumerate((b0, b1)):
                        sl = slice(h * NS, (h + 1) * NS)
                        xo = xs_v[:, b_, mc * 65:(mc + 1) * 65]
                        nc.tensor.matmul(ppvs[h][:], xo, es[:, sl],
                                         start=(mc == 0), stop=(mc == NT - 1))

                for h, b_ in enumerate((b0, b1)):
                    nc.vector.tensor_copy(zrow[0:1, b_ * NS:(b_ + 1) * NS],
                                          ppvs[h][64:65, :])
                nc.sync.dma_start(
                    ztt[2 * pr:2 * pr + 2, :],
                    zrow[0:1, 2 * pr * NS:(2 * pr + 2) * NS]
                    .rearrange("one (b n) -> one b n", b=2))
                nc.vector.reciprocal(rzt[:bt, :], ztt[:bt, :])
                for h, b_ in enumerate((b0, b1)):
                    rb = rhp.tile([64, NS], F32, tag="rb")
                    nc.sync.dma_start(rb[:], rzt[b_:b_ + 1, :].unsqueeze(1)
                                      .broadcast_to([1, 64, NS]))
                    nc.vector.tensor_tensor(out=xgT_v[64:128, b_, :],
                                            in0=ppvs[h][0:64, :], in1=rb[:],
                                            op=ALU.mult)

        # ---- apply: per-node matmuls, fp16, batched 8 nodes per PSUM tile ----
        out_v = out_d.rearrange("(b n) o -> b (n o)", b=bt)
        with tc.tile_pool(name="pap", bufs=6, space="PSUM") as pap, \
             tc.tile_pool(name="oap", bufs=4) as oap:
            for nb in range(NS // 8):
                pA = pap.tile([16, 8 * DO], F32, tag="pA")
                for jj in range(8):
                    n_ = nb * 8 + jj
                    nc.tensor.matmul(pA[:bt, jj * DO:(jj + 1) * DO],
                                     xgT_v[:, :, n_], W_v[:, :, n_],
                                     start=True, stop=True)
                ob = oap.tile([16, 8 * DO], F32, tag="ob")
                nc.vector.tensor_tensor(out=ob[:bt, :], in0=pA[:bt, :],
                                        in1=biasR[:bt, :], op=ALU.add)
                nc.scalar.dma_start(
                    out_v[:, nb * 8 * DO:(nb + 1) * 8 * DO], ob[:bt, :])

    nc.compile()
    return nc


_NC_CACHE = {}


def _get_nc():
    if "nc" not in _NC_CACHE:
        _NC_CACHE["nc"] = build_program()
    return _NC_CACHE["nc"]


def make_in_maps(x, node_embeddings, time_embeddings, weights_pool, bias_pool,
                 N=NFULL, bt=BT, n_cores=N_CORES):
    NS = N // n_cores
    NT = N // 128
    # x_r: [bt, mc, p, c] -> [p, bt, mc, c] -> rows (b*128+p), 1040 cols
    x4 = np.concatenate(
        [x.reshape(bt, N, DI).astype(np.float16),
         np.ones((bt, N, 1), np.float16)], axis=2)      # (bt, N, 65)
    x_r = np.ascontiguousarray(
        x4.reshape(bt, NT, 128, DI + 1).transpose(0, 2, 1, 3)
        .reshape(bt * 128, NT * (DI + 1)))
    ne = node_embeddings.astype(np.float32)
    neT = np.ascontiguousarray(ne.T)                    # (64, N)
    te = np.ascontiguousarray(time_embeddings.reshape(bt, D).astype(np.float32))
    wpf = np.ascontiguousarray(
        weights_pool.reshape(D, 2 * DI * DO).astype(np.float32))
    bpf = np.ascontiguousarray(bias_pool.astype(np.float32))
    x3 = x.reshape(bt, N, DI)
    maps = []
    for c in range(n_cores):
        sl = slice(c * NS, (c + 1) * NS)
        xshT = np.ascontiguousarray(
            x3[:, sl, :].astype(np.float16).transpose(2, 0, 1)
            .reshape(DI, bt * NS))
        maps.append({
            "x_r": x_r,
            "xshT": xshT,
            "neT": neT,
            "neTs": np.ascontiguousarray(neT[:, sl]),
            "te": te,
            "wp": wpf,
            "bp": bpf,
        })
    return maps


def kernel(x, node_embeddings, time_embeddings, weights_pool, bias_pool,
           ln_gamma=None, ln_beta=None, _trace=False):
    nc = _get_nc()
    in_maps = make_in_maps(np.asarray(x), np.asarray(node_embeddings),
                           np.asarray(time_embeddings),
                           np.asarray(weights_pool), np.asarray(bias_pool))
    res = run_bass_kernel_spmd(nc, in_maps, core_ids=list(range(N_CORES)),
                               trace=_trace)
    NS = NFULL // N_CORES
    shards = [res.results[c]["out_sh"].reshape(B, T, NS, DO)
              for c in range(N_CORES)]
    out = np.concatenate(shards, axis=2)
    if _trace:
        kernel._last_results = res
    return out
